# revision 16
# baseline (speedup 1.0000x reference)
"""Trainium2 Bass kernel for a two-window sparse causal self-attention block.

Model (B=2, T=2048, C=1024):
  - 8 "short" heads: d_qk=32,  window 256
  - 8 "long"  heads: d_qk=128, window 1024
  - value/output head dim 64, output projection C x C.

Sharding (8 cores): data-parallel over batch (2) x head-parallel over head
groups (4). Core c = 4*b + g handles batch b and heads {2g, 2g+1} of both the
short and long sets. Each core computes its 4 heads' attention plus the
corresponding 256 rows of Wproj, producing a partial [T, C] output; the host
sums the 4 partials per batch element.

Device-side design notes:
  - float32r matmuls everywhere: full PE rate (1 cycle/row at N>=256) vs 2
    cycles/row for fp32, ~1.5e-4 matmul relative error.
  - everything is computed in "transposed" orientation so no on-device
    transposes are needed: host passes xT [C, T]; projections give qT/kT
    [d, T] and v [T, dv]; scores sT[k, q] = kT.T @ qT; yT[dv, q] = v_aug.T @
    pT with a ones column in v so row 64 of yT accumulates softmax sums.
  - queries processed in groups of 512 (4 blocks) so score/AV matmuls run at
    N=512; the causal band mask is applied multiplicatively on exp(scores)
    using 512-wide sliding windows into a host-precomputed [128, W+896] band
    image.
  - exp skips the max-subtraction: inputs are well-scaled (|scores| < ~10).
  - normalization: reciprocal of the sums row, broadcast across partitions
    via a rank-1 matmul against a ones row, multiply into the yT tiles.
"""

import math

import numpy as np

import concourse.bass as bass
import concourse.mybir as mybir
import concourse.tile as tile
from concourse.bass_utils import run_bass_kernel_spmd

F32 = mybir.dt.float32
F32R = mybir.dt.float32r

B, T, C = 2, 2048, 1024
HS, DS = 8, 32
HL, DL = 8, 128
HD = 64
WIN_S, WIN_L = 256, 1024
NT = T // 128    # 16 t-blocks
NCB = C // 128   # 8 c-blocks
NG = T // 512    # 4 query groups
VW = HD + 1      # v columns + ones column for softmax sums
N_CORES = 8


def _split_waits(nc: bass.Bass) -> int:
    """Walrus in this env accepts at most 1 sync wait per instruction.
    Hoist extra waits onto same-engine InstNoOp instructions placed just
    before the owning instruction (same-engine program order preserves the
    blocking semantics)."""
    import bass_rust

    n_added = 0
    for f in nc.m.functions:
        for bb in f.blocks:
            insts = bb.instructions
            if not any(inst.sync_info and len(inst.sync_info.on_wait) > 1
                       for inst in insts):
                continue
            new = []
            for inst in insts:
                si = inst.sync_info
                waits = list(si.on_wait) if si else []
                if len(waits) > 1:
                    for i, w in enumerate(waits[:-1]):
                        nop = mybir.InstNoOp(
                            name=f"{inst.name}_hw{i}",
                            sync_info=bass_rust.SyncInfo(on_wait=[w], on_update=[]),
                            bass_nofuse=True,
                            engine=inst.engine,
                        )
                        new.append(nop)
                        n_added += 1
                    inst.sync_info = bass_rust.SyncInfo(
                        on_wait=waits[-1:], on_update=list(si.on_update))
                new.append(inst)
            bb.instructions = new
    return n_added


def _patch_tile_drain():
    """This walrus build rejects >1 sync wait on the TileContext tail drain
    ("Too many sync wait commands"). Re-emit the drain's waits as individual
    wait_ge instructions on the sync engine."""
    import bass_rust
    from concourse.tile import ScopedClock, TileContext

    def _drain_and_barrier(self, tick_clock, wait_clock):
        nc = self.nc
        drain_inst = nc.sync.drain()
        wait_clock.add_sem_waits(
            drain_inst.ins, ScopedClock({None: tick_clock.global_clock})
        )
        si = drain_inst.ins.sync_info
        waits = list(si.on_wait) if si is not None else []
        if len(waits) > 1:
            drain_inst.ins.sync_info = bass_rust.SyncInfo(on_wait=[], on_update=[])
            sems = {h.name: h for h in self.sems.allocated().values()}
            for w in waits:
                nc.sync.wait_ge(sems[w.ant_name], w.wait_value)
        nc.all_engine_barrier()
        popped = nc._tile_sem_poison_stack.pop()
        assert popped is self._sem_poison
        nc.clear_and_free_semaphores(list(self.sems.allocated().values()))
        nc.all_engine_barrier()

    TileContext._drain_and_barrier = _drain_and_barrier


_patch_tile_drain()


def _build_program() -> bass.Bass:
    nc = bass.Bass()

    xt_d = nc.dram_tensor("xt", [C, T], F32, kind="ExternalInput")
    wsqk_d = nc.dram_tensor("wsqk", [C, 128], F32, kind="ExternalInput")
    wql_d = nc.dram_tensor("wql", [C, 256], F32, kind="ExternalInput")
    wkl_d = nc.dram_tensor("wkl", [C, 256], F32, kind="ExternalInput")
    wv_d = nc.dram_tensor("wv", [C, 256], F32, kind="ExternalInput")
    wp_d = nc.dram_tensor("wp", [256, C], F32, kind="ExternalInput")
    bs_d = nc.dram_tensor("band_s", [128, WIN_S + 896], F32, kind="ExternalInput")
    bl_d = nc.dram_tensor("band_l", [128, WIN_L + 896], F32, kind="ExternalInput")
    ones_d = nc.dram_tensor("ones", [128, 64], F32, kind="ExternalInput")
    out_d = nc.dram_tensor("out", [T, C], F32, kind="ExternalOutput")

    scale_s = 1.0 / math.sqrt(DS)
    scale_l = 1.0 / math.sqrt(DL)

    with tile.TileContext(nc) as tc:
        with (
            tc.tile_pool(name="const", bufs=1) as const,
            tc.tile_pool(name="qkp", bufs=1) as qkp,
            tc.tile_pool(name="vp", bufs=1) as vp,
            tc.tile_pool(name="bigps", bufs=2, space="PSUM") as bigps,
            tc.tile_pool(name="stps", bufs=3, space="PSUM") as stps,
            tc.tile_pool(name="yhps", bufs=2, space="PSUM") as yhps,
            tc.tile_pool(name="rbps", bufs=1, space="PSUM") as rbps,
        ):
            # ---- weights (f32r views of the fp32 DRAM data) ----
            wsqk = const.tile([128, NCB, 128], F32R, tag="wsqk", name="wsqk")
            nc.sync.dma_start(wsqk[:], wsqk_d[:, :].bitcast(F32R).rearrange("(cb p) d -> p cb d", p=128))
            wql = const.tile([128, NCB, 256], F32R, tag="wql", name="wql")
            nc.sync.dma_start(wql[:], wql_d[:, :].bitcast(F32R).rearrange("(cb p) d -> p cb d", p=128))
            wkl = const.tile([128, NCB, 256], F32R, tag="wkl", name="wkl")
            nc.sync.dma_start(wkl[:], wkl_d[:, :].bitcast(F32R).rearrange("(cb p) d -> p cb d", p=128))
            wv = const.tile([128, NCB, 256], F32R, tag="wv", name="wv")
            nc.sync.dma_start(wv[:], wv_d[:, :].bitcast(F32R).rearrange("(cb p) d -> p cb d", p=128))

            # ---- projection outputs (persist across both stages) ----
            # short heads: qts/kts [64, T], rows 0-31 head0, 32-63 head1
            # (separate tiles so score matmul lhsT/rhs base partitions align)
            qts = qkp.tile([64, T], F32R, tag="qts", name="qts")
            kts = qkp.tile([64, T], F32R, tag="kts", name="kts")
            qtl = [qkp.tile([128, T], F32R, tag=f"qtl{h}", name=f"qtl{h}") for h in range(2)]
            ktl = [qkp.tile([128, T], F32R, tag=f"ktl{h}", name=f"ktl{h}") for h in range(2)]
            # v tiles per head, [128, NT*VW]; col 64 of each block = 1.0
            vt = [vp.tile([128, NT * VW], F32R, tag=f"vt{i}", name=f"vt{i}") for i in range(4)]

            # ================= stage A: projections =================
            with tc.tile_pool(name="xtp", bufs=1) as xtp:
                xt = []
                for cb in range(NCB):
                    t_ = xtp.tile([128, T], F32R, tag=f"xt{cb}", name=f"xt{cb}")
                    nc.sync.dma_start(t_[:], xt_d[cb * 128:(cb + 1) * 128, :].bitcast(F32R))
                    xt.append(t_)

                proj_jobs = [(wsqk, None, None)]
                for h in range(2):
                    proj_jobs.append((wql, h, qtl[h]))
                    proj_jobs.append((wkl, h, ktl[h]))
                for w, h, dst in proj_jobs:
                    for tch in range(T // 512):
                        ps = bigps.tile([128, 512], F32, tag="bigps", name="bigps")
                        for cb in range(NCB):
                            lhsT = w[:, cb, :] if h is None else w[:, cb, h * 128:(h + 1) * 128]
                            nc.tensor.matmul(
                                ps[:], lhsT, xt[cb][:, tch * 512:(tch + 1) * 512],
                                start=(cb == 0), stop=(cb == NCB - 1),
                            )
                        sl = (slice(None), slice(tch * 512, (tch + 1) * 512))
                        if dst is None:
                            nc.vector.tensor_copy(qts[sl], ps[0:64, :])
                            nc.vector.tensor_copy(kts[sl], ps[64:128, :])
                        else:
                            nc.vector.tensor_copy(dst[sl], ps[:])

                for tb in range(NT):
                    ps = bigps.tile([128, 512], F32, tag="bigps", name="bigps")
                    for cb in range(NCB):
                        nc.tensor.matmul(
                            ps[:, 0:256], xt[cb][:, tb * 128:(tb + 1) * 128], wv[:, cb, :],
                            start=(cb == 0), stop=(cb == NCB - 1),
                        )
                    for i in range(4):
                        nc.vector.tensor_copy(
                            vt[i][:, tb * VW: tb * VW + HD], ps[:, i * 64:(i + 1) * 64]
                        )

            # ============ stage B: attention + output projection ============
            with (
                tc.tile_pool(name="attnc", bufs=1) as attnc,
                tc.tile_pool(name="ptp", bufs=4) as ptp,
                tc.tile_pool(name="ytp", bufs=2) as ytp,
                tc.tile_pool(name="obp", bufs=3) as obp,
                tc.tile_pool(name="smallp", bufs=2) as smallp,
            ):
                wp0 = attnc.tile([128, C], F32R, tag="wp0", name="wp0")
                nc.sync.dma_start(wp0[:], wp_d[0:128, :].bitcast(F32R))
                wp1 = attnc.tile([128, C], F32R, tag="wp1", name="wp1")
                nc.sync.dma_start(wp1[:], wp_d[128:256, :].bitcast(F32R))
                band_s = attnc.tile([128, WIN_S + 896], F32R, tag="band_s", name="band_s")
                nc.sync.dma_start(band_s[:], bs_d[:, :].bitcast(F32R))
                band_l = attnc.tile([128, WIN_L + 896], F32R, tag="band_l", name="band_l")
                nc.sync.dma_start(band_l[:], bl_d[:, :].bitcast(F32R))
                ones64 = attnc.tile([1, 64], F32R, tag="ones64", name="ones64")
                nc.sync.dma_start(ones64[:], ones_d[0:1, :].bitcast(F32R))
                # ones column of each v block (strided view [:, 64::65])
                for i in range(4):
                    v3 = vt[i][:, :].rearrange("p (nt vw) -> p nt vw", vw=VW)
                    nc.sync.dma_start(v3[:, :, HD], ones_d[:, 0:NT].bitcast(F32R))

                for qg in range(NG):
                    q0 = qg * 512
                    yts = [ytp.tile([128, 512], F32R, tag=f"yts{i}", name=f"yts{i}")
                           for i in range(2)]

                    heads = []
                    for h in range(2):  # short heads
                        heads.append((
                            lambda kb, h=h: kts[32 * h: 32 * h + 32, kb * 128:(kb + 1) * 128],
                            qts[32 * h: 32 * h + 32, q0: q0 + 512],
                            vt[h], WIN_S, scale_s, band_s, yts[0], 64 * h,
                        ))
                    for h in range(2):  # long heads
                        heads.append((
                            lambda kb, h=h: ktl[h][:, kb * 128:(kb + 1) * 128],
                            qtl[h][:, q0: q0 + 512],
                            vt[2 + h], WIN_L, scale_l, band_l, yts[1], 64 * h,
                        ))

                    for kt_ap, qt_ap, v_tile, win, scale, band, dest, poff in heads:
                        kb_lo = max(0, q0 - win) // 128
                        kb_hi = (q0 + 384) // 128
                        kbs = list(range(kb_lo, kb_hi + 1))
                        yh = yhps.tile([VW, 512], F32, tag="yh", name="yh")
                        pts = []
                        for kb in kbs:
                            st = stps.tile([128, 512], F32, tag="st", name="st")
                            nc.tensor.matmul(st[:], kt_ap(kb), qt_ap, start=True, stop=True)
                            pt = ptp.tile([128, 512], F32R, tag="pt", name="pt")
                            nc.scalar.activation(
                                pt[:], st[:], mybir.ActivationFunctionType.Exp, scale=scale
                            )
                            delta = kb * 128 - q0
                            if not (512 - win <= delta <= -128):
                                off = 384 - delta
                                nc.vector.tensor_mul(pt[:], pt[:], band[:, off: off + 512])
                            pts.append(pt)
                        for i, (kb, pt) in enumerate(zip(kbs, pts)):
                            nc.tensor.matmul(
                                yh[:], v_tile[:, kb * VW:(kb + 1) * VW], pt[:],
                                start=(i == 0), stop=(i == len(pts) - 1),
                            )
                        r = smallp.tile([1, 512], F32R, tag="r", name="r")
                        with nc.allow_low_precision(reason="f32r rounding of softmax sums"):
                            nc.vector.reciprocal(r[:], yh[HD: HD + 1, :])
                        rb = rbps.tile([64, 512], F32, tag="rb", name="rb")
                        nc.tensor.matmul(rb[:], ones64[:], r[:], start=True, stop=True)
                        rbs = smallp.tile([64, 512], F32, tag="rbs", name="rbs")
                        nc.vector.tensor_copy(rbs[:], rb[:])
                        with nc.allow_low_precision(reason="f32r rounding of attn out"):
                            nc.vector.tensor_mul(dest[poff: poff + 64, :], yh[0:HD, :], rbs[:])

                    for sub in range(4):
                        qs = q0 + sub * 128
                        ssl = (slice(None), slice(sub * 128, (sub + 1) * 128))
                        for nh in range(2):
                            po = bigps.tile([128, 512], F32, tag="bigps", name="bigps")
                            nc.tensor.matmul(po[:], yts[0][ssl], wp0[:, nh * 512:(nh + 1) * 512],
                                             start=True, stop=False)
                            nc.tensor.matmul(po[:], yts[1][ssl], wp1[:, nh * 512:(nh + 1) * 512],
                                             start=False, stop=True)
                            ob = obp.tile([128, 512], F32, tag="ob", name="ob")
                            nc.vector.tensor_copy(ob[:], po[:])
                            nc.sync.dma_start(out_d[qs: qs + 128, nh * 512:(nh + 1) * 512], ob[:])

    return nc


_PROGRAM = None


def _get_program() -> bass.Bass:
    global _PROGRAM
    if _PROGRAM is None:
        _PROGRAM = _build_program()
        _split_waits(_PROGRAM)
    return _PROGRAM


def _band_image(win: int) -> np.ndarray:
    """[128, win+896] 0/1 image: B[r, u] = 1 iff (u - 384 - r) in [0, win)."""
    u = np.arange(win + 896)[None, :]
    r = np.arange(128)[:, None]
    d = u - 384 - r
    return ((d >= 0) & (d < win)).astype(np.float32)


def make_in_maps(x, Wqk_short, Wv_short, Wqk_long, Wv_long, Wproj):
    """Host-side sharding: per-core input dict for core c = 4*b + g."""
    x = np.ascontiguousarray(np.asarray(x, dtype=np.float32))
    Wqk_short = np.asarray(Wqk_short, dtype=np.float32)
    Wv_short = np.asarray(Wv_short, dtype=np.float32)
    Wqk_long = np.asarray(Wqk_long, dtype=np.float32)
    Wv_long = np.asarray(Wv_long, dtype=np.float32)
    Wproj = np.asarray(Wproj, dtype=np.float32)
    assert x.shape == (B, T, C)

    xts = [np.ascontiguousarray(x[b].T) for b in range(B)]
    band_s = _band_image(WIN_S)
    band_l = _band_image(WIN_L)
    ones = np.ones((128, 64), dtype=np.float32)
    in_maps = []
    for c in range(N_CORES):
        b, g = divmod(c, 4)
        wsqk = np.ascontiguousarray(np.concatenate(
            [Wqk_short[:, g * 64:(g + 1) * 64],
             Wqk_short[:, 256 + g * 64: 256 + (g + 1) * 64]], axis=1))
        wql = np.ascontiguousarray(Wqk_long[:, g * 256:(g + 1) * 256])
        wkl = np.ascontiguousarray(Wqk_long[:, 1024 + g * 256: 1024 + (g + 1) * 256])
        wv = np.ascontiguousarray(np.concatenate(
            [Wv_short[:, g * 128:(g + 1) * 128],
             Wv_long[:, g * 128:(g + 1) * 128]], axis=1))
        wp = np.ascontiguousarray(np.concatenate(
            [Wproj[g * 128:(g + 1) * 128, :],
             Wproj[512 + g * 128: 512 + (g + 1) * 128, :]], axis=0))
        in_maps.append({
            "xt": xts[b], "wsqk": wsqk, "wql": wql, "wkl": wkl, "wv": wv, "wp": wp,
            "band_s": band_s, "band_l": band_l, "ones": ones,
        })
    return in_maps


def gather(results) -> np.ndarray:
    out = np.empty((B, T, C), dtype=np.float32)
    for b in range(B):
        acc = np.zeros((T, C), dtype=np.float64)
        for g in range(4):
            acc += results[4 * b + g]["out"]
        out[b] = acc.astype(np.float32)
    return out


def kernel(x, Wqk_short, Wv_short, Wqk_long, Wv_long, Wproj, **run_kwargs):
    nc = _get_program()
    in_maps = make_in_maps(x, Wqk_short, Wv_short, Wqk_long, Wv_long, Wproj)
    res = run_bass_kernel_spmd(nc, in_maps, core_ids=list(range(N_CORES)), **run_kwargs)
    out = gather(res.results)
    if run_kwargs:
        kernel.last_results = res
    return out
